# revision 12
# baseline (speedup 1.0000x reference)
"""Distributed multi-head attention kernel for 8 TRN2 NeuronCores.

Problem: B=2, N=2048, C=1024, H=16 heads, D=64.
  out = softmax((q@Wq)(k@Wk)^T / sqrt(D)) @ (v@Wv) @ Wo   (biases are zero)

Sharding (hybrid): data-parallel over batch x tensor-parallel over heads.
Core c owns batch b=c//4 and head group g=c%4 (4 heads = 256 channels); it
projects Q/K/V for its own 256 columns over the full sequence, runs
attention for its 4 heads, applies its 256 rows of Wo and writes a PARTIAL
(1024, 2048) bf16 output.  The host sums the 4 partials per batch (the
out_proj all-reduce) and transposes.  Zero redundant FLOPs.

ScalarE is the critical path (~145us of exp ACTIVATE at (1024+352)/1.2 ns
per [128,1024] tile).  The emission strategy keeps it saturated:
  - batched DMAs (one descriptor per tensor half) so the head is
    transfer-bound, not dispatch-bound;
  - every non-score PE task (V' projection, PV, out_proj) is queued as
    small filler items interleaved into the score/exp slot stream, with
    slot gates to avoid in-order PE stalls on not-yet-ready deps;
  - out_proj of query block qb is spread into the unit after fin((qb,hp1));
  - the last unit's PV runs head-even first so its normalize overlaps the
    head-odd chain.
"""

import sys

sys.path.insert(0, "/opt/trn_rl_repo")

from contextlib import ExitStack

import numpy as np
import ml_dtypes

import concourse.bass as bass
import concourse.bacc as bacc
import concourse.mybir as mybir
import concourse.tile as tile
from concourse.bass_utils import run_bass_kernel_spmd

BF16 = mybir.dt.bfloat16
F32 = mybir.dt.float32
Exp = mybir.ActivationFunctionType.Exp

B, N, C = 2, 2048, 1024
H, D = 16, 64
HL, CL = 4, 256     # heads / channels owned per core
DV = D + 1          # V' columns per head incl. ones column
NCH = N // 128      # 16 key chunks
SCALE = 1.0 / np.sqrt(D)

_CACHE = {}


def build_nc():
    nc = bacc.Bacc("TRN2", target_bir_lowering=False, debug=False, num_devices=8)

    xqT = nc.declare_dram_parameter("xqT", [C, N], BF16, isOutput=False)
    xkT = nc.declare_dram_parameter("xkT", [C, N], BF16, isOutput=False)
    xvT = nc.declare_dram_parameter("xvT", [C, N], BF16, isOutput=False)
    wq = nc.declare_dram_parameter("wq", [C, CL], BF16, isOutput=False)
    wk = nc.declare_dram_parameter("wk", [C, CL], BF16, isOutput=False)
    wv = nc.declare_dram_parameter("wv", [C, CL], BF16, isOutput=False)
    wo = nc.declare_dram_parameter("wo", [CL, C], BF16, isOutput=False)
    outT = nc.declare_dram_parameter("outT", [C, N], BF16, isOutput=True)

    with tile.TileContext(nc) as tc, ExitStack() as top:
        res = top.enter_context(tc.tile_pool(name="res", bufs=1))
        qT_sb = res.tile([128, 2 * N], BF16, tag="qT")   # chunk hp at cols N*hp
        kT_sb = res.tile([128, 2 * N], BF16, tag="kT")
        v1_sb = res.tile([128, NCH * HL * DV], BF16, tag="v1")
        aT_sb = res.tile([128, 2 * N], BF16, tag="aT")
        v4 = v1_sb[:].rearrange("p (kc h x) -> p kc h x", kc=NCH, x=DV)

        def dma_in_grouped(dst_tile, src, blocks, tag):
            """One DMA per row-block: src rows [128*b0, 128*b1) -> dst col
            groups (dst-local), dst laid out [128, nblk*width]."""
            width = src.shape[1]
            dst3 = dst_tile[:].rearrange("p (c m) -> p c m", m=width)
            for b0, b1 in blocks:
                nc.sync.dma_start(
                    out=dst3[:, 0:b1 - b0, :],
                    in_=src[128 * b0:128 * b1, :]
                    .rearrange("(c p) m -> p c m", p=128))

        # ---------------- phase A: K and Q projections ----------------
        with ExitStack() as ph:
            wpool = ph.enter_context(tc.tile_pool(name="wpool", bufs=1))
            xpool = ph.enter_context(tc.tile_pool(name="xpool", bufs=1))
            qkps = ph.enter_context(tc.tile_pool(name="qkps", bufs=4, space="PSUM"))

            # preload the exp table set so the first real ACTIVATE is cheap
            dummy = wpool.tile([1, 8], F32, tag="dummy")
            nc.vector.memset(dummy[:], 0.0)
            nc.scalar.activation(dummy[:], dummy[:], Exp, scale=1.0)

            wk_t = wpool.tile([128, 8 * CL], BF16, tag="wk")
            wq_t = wpool.tile([128, 8 * CL], BF16, tag="wq")
            # x tensors as quarter tiles (2 chunks each): dep tracking is
            # per-tile, so finer tiles let projections start on early data.
            xk_q = [xpool.tile([128, 4 * N], BF16, tag=f"xk{i}", name=f"xk_q{i}") for i in range(2)]
            xq_q = [xpool.tile([128, 4 * N], BF16, tag=f"xq{i}", name=f"xq_q{i}") for i in range(2)]
            dma_in_grouped(wk_t, wk, [(0, 8)], "wk")
            dma_in_grouped(wq_t, wq, [(0, 8)], "wq")
            for i in range(2):
                dma_in_grouped(xk_q[i], xkT, [(4 * i, 4 * i + 4)], f"xk{i}")
            for i in range(2):
                dma_in_grouped(xq_q[i], xqT, [(4 * i, 4 * i + 4)], f"xq{i}")
            wk3 = wk_t[:].rearrange("p (c m) -> p c m", m=CL)
            wq3 = wq_t[:].rearrange("p (c m) -> p c m", m=CL)

            def xsl(xq_list, cc):
                return xq_list[cc // 4][:].rearrange(
                    "p (c n) -> p c n", n=N)[:, cc % 4, :]

            def proj_qk(dst_sb, w3, x_list, nm):
                for hp in range(2):
                    ps = [qkps.tile([128, 512], F32, tag="ps", name=f"{nm}ps{hp}_{qb}")
                          for qb in range(4)]
                    for cc in range(8):
                        xc = xsl(x_list, cc)
                        for qb in range(4):
                            nc.tensor.matmul(ps[qb][:],
                                             w3[:, cc, 128 * hp:128 * (hp + 1)],
                                             xc[:, 512 * qb:512 * (qb + 1)],
                                             start=(cc == 0), stop=(cc == 7))
                    for qb in range(4):
                        nc.vector.tensor_copy(
                            dst_sb[:, N * hp + 512 * qb:N * hp + 512 * (qb + 1)],
                            ps[qb][:])

            proj_qk(kT_sb, wk3, xk_q, "k")
            proj_qk(qT_sb, wq3, xq_q, "q")

        # ---------------- attention-phase pools ----------------
        P_pool = top.enter_context(tc.tile_pool(name="Ppool", bufs=2))
        spool = top.enter_context(tc.tile_pool(name="spool", bufs=2, space="PSUM"))
        popool = top.enter_context(tc.tile_pool(name="popool", bufs=2, space="PSUM"))
        wvpool = top.enter_context(tc.tile_pool(name="wvpool", bufs=1))
        xvpool = top.enter_context(tc.tile_pool(name="xvpool", bufs=1))
        wopool = top.enter_context(tc.tile_pool(name="wopool", bufs=1))
        evpool = top.enter_context(tc.tile_pool(name="evpool", bufs=2))
        dpool = top.enter_context(tc.tile_pool(name="dpool", bufs=2))

        wv_t = wvpool.tile([128, 8 * CL], BF16, tag="wv")
        xv_q = [xvpool.tile([128, 4 * N], BF16, tag=f"xv{i}", name=f"xv_q{i}") for i in range(2)]
        wo_t = wopool.tile([128, 2 * C], BF16, tag="wo")
        dma_in_grouped(wv_t, wv, [(0, 8)], "wv")
        for i in range(2):
            dma_in_grouped(xv_q[i], xvT, [(4 * i, 4 * i + 4)], f"xv{i}")
        dma_in_grouped(wo_t, wo, [(0, 2)], "wo")
        wv3 = wv_t[:].rearrange("p (c m) -> p c m", m=CL)
        wo3 = wo_t[:].rearrange("p (c m) -> p c m", m=C)
        nc.vector.memset(v4[:, :, :, D:DV], 1.0)

        # --- filler queue: (cost, min_slot, emit_fn) consumed per slot ---
        fillers = []

        def emit_fillers(slot, budget=3):
            while fillers and budget > 0:
                cost, min_slot, fn = fillers[0]
                if slot < min_slot:
                    return
                fillers.pop(0)
                fn()
                budget -= cost

        def vproj_items(kc):
            """Two filler halves per key chunk (finer PE granularity)."""
            box = {}

            def fn1():
                ps = popool.tile([128, 512], F32, tag="out", name=f"vps{kc}")
                box["ps"] = ps
                for cc in range(4):
                    nc.tensor.matmul(ps[:, 0:CL],
                                     xsl(xv_q, cc)[:, 128 * kc:128 * (kc + 1)],
                                     wv3[:, cc, :],
                                     start=(cc == 0), stop=False)

            def fn2():
                ps = box["ps"]
                for cc in range(4, 8):
                    nc.tensor.matmul(ps[:, 0:CL],
                                     xsl(xv_q, cc)[:, 128 * kc:128 * (kc + 1)],
                                     wv3[:, cc, :],
                                     start=False, stop=(cc == 7))
                nc.vector.tensor_copy(
                    v4[:, kc, :, 0:D],
                    ps[:, 0:CL].rearrange("p (h d) -> p h d", d=D))
            return [(2, 1, fn1), (2, 1, fn2)]

        def pv_item(u, Pp, po, par, kc):
            qb, hp = u // 2, u % 2

            def fn():
                nc.tensor.matmul(po[0:DV, :], v4[:, kc, 2 * hp + par, :],
                                 Pp[:, 1024 * kc + 512 * par:
                                    1024 * kc + 512 * (par + 1)],
                                 start=(kc == 0), stop=(kc == NCH - 1))
            return (1, 2, fn)

        def out_item(qb, ev, m):
            def fn():
                ps = popool.tile([128, 512], F32, tag="out", name=f"ops{qb}_{m}")
                for lc in range(2):
                    nc.tensor.matmul(ps[:], wo3[:, lc, 128 * m:128 * (m + 1)],
                                     aT_sb[:, N * lc + 512 * qb:
                                           N * lc + 512 * (qb + 1)],
                                     start=(lc == 0), stop=(lc == 1))
                ev3 = ev[:].rearrange("p (m n) -> p m n", n=512)
                nc.vector.tensor_copy(ev3[:, m, :], ps[:])
            return (2, 2, fn)

        def out_flush(qb, ev):
            def fn():
                nc.sync.dma_start(
                    out=outT.rearrange("(m p) n -> p m n", p=128)
                    [:, :, 512 * qb:512 * (qb + 1)],
                    in_=ev[:].rearrange("p (m n) -> p m n", n=512))
            return (0, 2, fn)

        # exp offload: these key chunks use the vector engine's Schraudolph
        # bit-trick (i16 = round(a*s + b) reinterpreted as bf16) instead of
        # ScalarE's ACTIVATE, balancing the two engines (~3% elementwise,
        # <0.1% on softmax output after num/denom cancellation).
        OFFLOAD_KC = (12, 14)
        LOG2E = float(np.log2(np.e))
        SCH_A = 128.0 * LOG2E * float(SCALE)
        SCH_B = 128.0 * (127.0 - 0.0436775)
        I16 = mybir.dt.int16

        def emit_scores_unit(u, with_fillers=True):
            qb, hp = u // 2, u % 2
            Pp = P_pool.tile([128, NCH * 1024], BF16, tag="P", name=f"P{u}")
            for kc in range(NCH):
                st = spool.tile([128, 1024], F32, tag="st", name=f"st{u}_{kc}")
                ksl = kT_sb[:, N * hp + 128 * kc:N * hp + 128 * (kc + 1)]
                qsl = qT_sb[:, N * hp + 512 * qb:N * hp + 512 * (qb + 1)]
                nc.tensor.matmul(st[:, 0:512], ksl[0:64, :], qsl[0:64, :],
                                 start=True, stop=True)
                nc.tensor.matmul(st[:, 512:1024], ksl[64:128, :], qsl[64:128, :],
                                 start=True, stop=True)
                pdst = Pp[:, 1024 * kc:1024 * (kc + 1)]
                if kc in OFFLOAD_KC:
                    nc.vector.tensor_scalar(pdst.bitcast(I16), st[:],
                                            SCH_A, SCH_B,
                                            mybir.AluOpType.mult,
                                            mybir.AluOpType.add)
                else:
                    nc.scalar.activation(pdst, st[:], Exp, scale=float(SCALE))
                if with_fillers:
                    emit_fillers(kc)
            return Pp

        def fin_head(u, po, par):
            qb, hp = u // 2, u % 2
            draw = dpool.tile([1, 512], F32, tag="draw", name=f"dr{u}_{par}")
            drow = dpool.tile([1, 512], F32, tag="drow", name=f"dw{u}_{par}")
            dinv = dpool.tile([64, 512], F32, tag="dinv", name=f"di{u}_{par}")
            nc.vector.tensor_copy(draw[:], po[D:DV, :])
            nc.vector.reciprocal_approx_fast(drow[:], draw[:])
            nc.gpsimd.partition_broadcast(dinv[:], drow[:])
            nc.vector.tensor_mul(
                aT_sb[64 * par:64 * (par + 1),
                      N * hp + 512 * qb:N * hp + 512 * (qb + 1)],
                po[0:D, :], dinv[:])

        # queue V' projection as fillers for scores(u0)/scores(u1) slots
        for kc in range(NCH):
            fillers.extend(vproj_items(kc))

        P_cur = emit_scores_unit(0)

        pending_out = None
        for u in range(8):
            qb, hp = u // 2, u % 2
            po_e = popool.tile([128, 512], F32, tag="po", name=f"poe{u}")
            po_o = popool.tile([128, 512], F32, tag="po", name=f"poo{u}")
            if u < 7:
                # PV(u) singles interleaved e,o per kc
                for kc in range(NCH):
                    fillers.append(pv_item(u, P_cur, po_e, 0, kc))
                    fillers.append(pv_item(u, P_cur, po_o, 1, kc))
                if pending_out is not None:
                    oqb, ev = pending_out
                    for m in range(8):
                        fillers.append(out_item(oqb, ev, m))
                    fillers.append(out_flush(oqb, ev))
                    pending_out = None
                P_cur = emit_scores_unit(u + 1)
                # drain any leftover fillers (PV must finish before fin)
                while fillers:
                    fillers.pop(0)[2]()
                fin_head(u, po_e, 0)
                fin_head(u, po_o, 1)
            else:
                # tail: head-even chain, normalize it while head-odd runs
                for kc in range(NCH):
                    pv_item(u, P_cur, po_e, 0, kc)[2]()
                fin_head(u, po_e, 0)
                for kc in range(NCH):
                    pv_item(u, P_cur, po_o, 1, kc)[2]()
                fin_head(u, po_o, 1)
            if hp == 1:
                ev = evpool.tile([128, 8 * 512], BF16, tag="ev", name=f"ev{qb}")
                if u < 7:
                    pending_out = (qb, ev)
                else:
                    for m in range(8):
                        out_item(qb, ev, m)[2]()
                    out_flush(qb, ev)[2]()

    nc.compile()
    return nc


def _get_nc():
    if "nc" not in _CACHE:
        _CACHE["nc"] = build_nc()
    return _CACHE["nc"]


def _make_in_maps(q, k, v, Wq, Wk, Wv, Wo):
    bf = ml_dtypes.bfloat16
    q, k, v = np.asarray(q), np.asarray(k), np.asarray(v)
    Wq, Wk, Wv, Wo = (np.asarray(a) for a in (Wq, Wk, Wv, Wo))
    xqT = [np.ascontiguousarray(q[b].T).astype(bf) for b in range(B)]
    xkT = [np.ascontiguousarray(k[b].T).astype(bf) for b in range(B)]
    xvT = [np.ascontiguousarray(v[b].T).astype(bf) for b in range(B)]
    in_maps = []
    for c in range(8):
        b, g = c // 4, c % 4
        cs = slice(CL * g, CL * (g + 1))
        in_maps.append({
            "xqT": xqT[b], "xkT": xkT[b], "xvT": xvT[b],
            "wq": np.ascontiguousarray(Wq[:, cs]).astype(bf),
            "wk": np.ascontiguousarray(Wk[:, cs]).astype(bf),
            "wv": np.ascontiguousarray(Wv[:, cs]).astype(bf),
            "wo": np.ascontiguousarray(Wo[cs, :]).astype(bf),
        })
    return in_maps


def _run(inputs, trace=False, **kw):
    nc = _get_nc()
    in_maps = _make_in_maps(inputs["q"], inputs["k"], inputs["v"],
                            inputs["Wq"], inputs["Wk"], inputs["Wv"], inputs["Wo"])
    res = None
    for attempt in range(3):
        try:
            res = run_bass_kernel_spmd(nc, in_maps, core_ids=list(range(8)),
                                       trace=trace, **kw)
            break
        except Exception:
            if attempt == 2:
                raise
            import time
            time.sleep(2.0)
    out = np.empty((B, N, C), np.float32)
    for b in range(B):
        acc = res.results[4 * b]["outT"].astype(np.float32)
        for g in range(1, 4):
            acc += res.results[4 * b + g]["outT"].astype(np.float32)
        out[b] = acc.T
    return out, res


def kernel(**inputs) -> np.ndarray:
    out, _ = _run(inputs, trace=False)
    return out


# revision 13
# speedup vs baseline: 1.0691x; 1.0691x over previous
"""Distributed multi-head attention kernel for 8 TRN2 NeuronCores.

Problem: B=2, N=2048, C=1024, H=16 heads, D=64.
  out = softmax((q@Wq)(k@Wk)^T / sqrt(D)) @ (v@Wv) @ Wo   (biases are zero)

Sharding (hybrid): data-parallel over batch x tensor-parallel over heads.
Core c owns batch b=c//4 and head group g=c%4 (4 heads = 256 channels); it
projects Q/K/V for its own 256 columns over the full sequence, runs
attention for its 4 heads, applies its 256 rows of Wo and writes a PARTIAL
(1024, 2048) bf16 output.  The host sums the 4 partials per batch (the
out_proj all-reduce) and transposes.  Zero redundant FLOPs.

ScalarE is the critical path (~145us of exp ACTIVATE at (1024+352)/1.2 ns
per [128,1024] tile).  The emission strategy keeps it saturated:
  - batched DMAs (one descriptor per tensor half) so the head is
    transfer-bound, not dispatch-bound;
  - every non-score PE task (V' projection, PV, out_proj) is queued as
    small filler items interleaved into the score/exp slot stream, with
    slot gates to avoid in-order PE stalls on not-yet-ready deps;
  - out_proj of query block qb is spread into the unit after fin((qb,hp1));
  - the last unit's PV runs head-even first so its normalize overlaps the
    head-odd chain.
"""

import sys

sys.path.insert(0, "/opt/trn_rl_repo")

from contextlib import ExitStack

import numpy as np
import ml_dtypes

import concourse.bass as bass
import concourse.bacc as bacc
import concourse.mybir as mybir
import concourse.tile as tile
from concourse.bass_utils import run_bass_kernel_spmd

BF16 = mybir.dt.bfloat16
F32 = mybir.dt.float32
Exp = mybir.ActivationFunctionType.Exp

B, N, C = 2, 2048, 1024
H, D = 16, 64
HL, CL = 4, 256     # heads / channels owned per core
DV = D + 1          # V' columns per head incl. ones column
NCH = N // 128      # 16 key chunks
SCALE = 1.0 / np.sqrt(D)

_CACHE = {}


def build_nc():
    nc = bacc.Bacc("TRN2", target_bir_lowering=False, debug=False, num_devices=8)

    xqT = nc.declare_dram_parameter("xqT", [C, N], BF16, isOutput=False)
    xkT = nc.declare_dram_parameter("xkT", [C, N], BF16, isOutput=False)
    xvT = nc.declare_dram_parameter("xvT", [C, N], BF16, isOutput=False)
    wq = nc.declare_dram_parameter("wq", [C, CL], BF16, isOutput=False)
    wk = nc.declare_dram_parameter("wk", [C, CL], BF16, isOutput=False)
    wv = nc.declare_dram_parameter("wv", [C, CL], BF16, isOutput=False)
    wo = nc.declare_dram_parameter("wo", [CL, C], BF16, isOutput=False)
    outT = nc.declare_dram_parameter("outT", [C, N], BF16, isOutput=True)

    with tile.TileContext(nc) as tc, ExitStack() as top:
        res = top.enter_context(tc.tile_pool(name="res", bufs=1))
        qT_sb = res.tile([128, 2 * N], BF16, tag="qT")   # chunk hp at cols N*hp
        kT_sb = res.tile([128, 2 * N], BF16, tag="kT")
        v1_sb = res.tile([128, NCH * HL * DV], BF16, tag="v1")
        aT_sb = res.tile([128, 2 * N], BF16, tag="aT")
        v4 = v1_sb[:].rearrange("p (kc h x) -> p kc h x", kc=NCH, x=DV)

        def dma_in_grouped(dst_tile, src, blocks, tag):
            """One DMA per row-block: src rows [128*b0, 128*b1) -> dst col
            groups (global block index), dst laid out [128, nblk*width]."""
            width = src.shape[1]
            dst3 = dst_tile[:].rearrange("p (c m) -> p c m", m=width)
            for b0, b1 in blocks:
                nc.sync.dma_start(
                    out=dst3[:, b0:b1, :],
                    in_=src[128 * b0:128 * b1, :]
                    .rearrange("(c p) m -> p c m", p=128))

        # ---------------- phase A: K and Q projections ----------------
        with ExitStack() as ph:
            wpool = ph.enter_context(tc.tile_pool(name="wpool", bufs=1))
            xpool = ph.enter_context(tc.tile_pool(name="xpool", bufs=1))
            qkps = ph.enter_context(tc.tile_pool(name="qkps", bufs=4, space="PSUM"))

            # preload the exp table set so the first real ACTIVATE is cheap
            dummy = wpool.tile([1, 8], F32, tag="dummy")
            nc.vector.memset(dummy[:], 0.0)
            nc.scalar.activation(dummy[:], dummy[:], Exp, scale=1.0)

            wk_t = wpool.tile([128, 8 * CL], BF16, tag="wk")
            wq_t = wpool.tile([128, 8 * CL], BF16, tag="wq")
            # x tensors as quarter tiles (2 chunks each): dep tracking is
            # per-tile, so finer tiles let projections start on early data.
            xk_t = xpool.tile([128, 8 * N], BF16, tag="xk")
            xq_t = xpool.tile([128, 8 * N], BF16, tag="xq")
            dma_in_grouped(wk_t, wk, [(0, 8)], "wk")
            dma_in_grouped(wq_t, wq, [(0, 8)], "wq")
            dma_in_grouped(xk_t, xkT, [(0, 4), (4, 8)], "xk")
            dma_in_grouped(xq_t, xqT, [(0, 4), (4, 8)], "xq")
            wk3 = wk_t[:].rearrange("p (c m) -> p c m", m=CL)
            wq3 = wq_t[:].rearrange("p (c m) -> p c m", m=CL)
            xk_q = [xk_t]
            xq_q = [xq_t]

            def xsl(x_list, cc):
                return x_list[0][:].rearrange(
                    "p (c n) -> p c n", n=N)[:, cc, :]

            def proj_qk(dst_sb, w3, x_list, nm):
                for hp in range(2):
                    ps = [qkps.tile([128, 512], F32, tag="ps", name=f"{nm}ps{hp}_{qb}")
                          for qb in range(4)]
                    for cc in range(8):
                        xc = xsl(x_list, cc)
                        for qb in range(4):
                            nc.tensor.matmul(ps[qb][:],
                                             w3[:, cc, 128 * hp:128 * (hp + 1)],
                                             xc[:, 512 * qb:512 * (qb + 1)],
                                             start=(cc == 0), stop=(cc == 7))
                    for qb in range(4):
                        nc.vector.tensor_copy(
                            dst_sb[:, N * hp + 512 * qb:N * hp + 512 * (qb + 1)],
                            ps[qb][:])

            proj_qk(kT_sb, wk3, xk_q, "k")
            proj_qk(qT_sb, wq3, xq_q, "q")

        # ---------------- attention-phase pools ----------------
        P_pool = top.enter_context(tc.tile_pool(name="Ppool", bufs=2))
        spool = top.enter_context(tc.tile_pool(name="spool", bufs=2, space="PSUM"))
        popool = top.enter_context(tc.tile_pool(name="popool", bufs=2, space="PSUM"))
        wvpool = top.enter_context(tc.tile_pool(name="wvpool", bufs=1))
        xvpool = top.enter_context(tc.tile_pool(name="xvpool", bufs=1))
        wopool = top.enter_context(tc.tile_pool(name="wopool", bufs=1))
        evpool = top.enter_context(tc.tile_pool(name="evpool", bufs=2))
        dpool = top.enter_context(tc.tile_pool(name="dpool", bufs=2))

        wv_t = wvpool.tile([128, 8 * CL], BF16, tag="wv")
        xv_t = xvpool.tile([128, 8 * N], BF16, tag="xv")
        wo_t = wopool.tile([128, 2 * C], BF16, tag="wo")
        dma_in_grouped(wv_t, wv, [(0, 8)], "wv")
        dma_in_grouped(xv_t, xvT, [(0, 4), (4, 8)], "xv")
        xv_q = [xv_t]
        dma_in_grouped(wo_t, wo, [(0, 2)], "wo")
        wv3 = wv_t[:].rearrange("p (c m) -> p c m", m=CL)
        wo3 = wo_t[:].rearrange("p (c m) -> p c m", m=C)
        nc.vector.memset(v4[:, :, :, D:DV], 1.0)

        # --- filler queue: (cost, min_slot, emit_fn) consumed per slot ---
        fillers = []

        def emit_fillers(slot, budget=3):
            while fillers and budget > 0:
                cost, min_slot, fn = fillers[0]
                if slot < min_slot:
                    return
                fillers.pop(0)
                fn()
                budget -= cost

        def vproj_items(kc):
            def fn():
                ps = popool.tile([128, 512], F32, tag="out", name=f"vps{kc}")
                for cc in range(8):
                    nc.tensor.matmul(ps[:, 0:CL],
                                     xsl(xv_q, cc)[:, 128 * kc:128 * (kc + 1)],
                                     wv3[:, cc, :],
                                     start=(cc == 0), stop=(cc == 7))
                nc.vector.tensor_copy(
                    v4[:, kc, :, 0:D],
                    ps[:, 0:CL].rearrange("p (h d) -> p h d", d=D))
            return [(4, 8, fn)]

        def pv_item(u, Pp, po, par, kc):
            qb, hp = u // 2, u % 2

            def fn():
                nc.tensor.matmul(po[0:DV, :], v4[:, kc, 2 * hp + par, :],
                                 Pp[:, 1024 * kc + 512 * par:
                                    1024 * kc + 512 * (par + 1)],
                                 start=(kc == 0), stop=(kc == NCH - 1))
            return (1, 2, fn)

        def out_item(qb, ev, m):
            def fn():
                ps = popool.tile([128, 512], F32, tag="out", name=f"ops{qb}_{m}")
                for lc in range(2):
                    nc.tensor.matmul(ps[:], wo3[:, lc, 128 * m:128 * (m + 1)],
                                     aT_sb[:, N * lc + 512 * qb:
                                           N * lc + 512 * (qb + 1)],
                                     start=(lc == 0), stop=(lc == 1))
                ev3 = ev[:].rearrange("p (m n) -> p m n", n=512)
                nc.vector.tensor_copy(ev3[:, m, :], ps[:])
            return (2, 2, fn)

        def out_flush(qb, ev):
            def fn():
                nc.sync.dma_start(
                    out=outT.rearrange("(m p) n -> p m n", p=128)
                    [:, :, 512 * qb:512 * (qb + 1)],
                    in_=ev[:].rearrange("p (m n) -> p m n", n=512))
            return (0, 2, fn)

        # exp offload: these key chunks use the vector engine's Schraudolph
        # bit-trick (i16 = round(a*s + b) reinterpreted as bf16) instead of
        # ScalarE's ACTIVATE, balancing the two engines (~3% elementwise,
        # <0.1% on softmax output after num/denom cancellation).
        OFFLOAD_KC = ()
        LOG2E = float(np.log2(np.e))
        SCH_A = 128.0 * LOG2E * float(SCALE)
        SCH_B = 128.0 * (127.0 - 0.0436775)
        I16 = mybir.dt.int16

        def emit_scores_unit(u, with_fillers=True):
            qb, hp = u // 2, u % 2
            Pp = P_pool.tile([128, NCH * 1024], BF16, tag="P", name=f"P{u}")
            for kc in range(NCH):
                st = spool.tile([128, 1024], F32, tag="st", name=f"st{u}_{kc}")
                ksl = kT_sb[:, N * hp + 128 * kc:N * hp + 128 * (kc + 1)]
                qsl = qT_sb[:, N * hp + 512 * qb:N * hp + 512 * (qb + 1)]
                nc.tensor.matmul(st[:, 0:512], ksl[0:64, :], qsl[0:64, :],
                                 start=True, stop=True)
                nc.tensor.matmul(st[:, 512:1024], ksl[64:128, :], qsl[64:128, :],
                                 start=True, stop=True)
                pdst = Pp[:, 1024 * kc:1024 * (kc + 1)]
                if kc in OFFLOAD_KC:
                    nc.vector.tensor_scalar(pdst.bitcast(I16), st[:],
                                            SCH_A, SCH_B,
                                            mybir.AluOpType.mult,
                                            mybir.AluOpType.add)
                else:
                    nc.scalar.activation(pdst, st[:], Exp, scale=float(SCALE))
                if with_fillers:
                    emit_fillers(kc)
            return Pp

        def fin_head(u, po, par):
            qb, hp = u // 2, u % 2
            draw = dpool.tile([1, 512], F32, tag="draw", name=f"dr{u}_{par}")
            drow = dpool.tile([1, 512], F32, tag="drow", name=f"dw{u}_{par}")
            dinv = dpool.tile([64, 512], F32, tag="dinv", name=f"di{u}_{par}")
            nc.vector.tensor_copy(draw[:], po[D:DV, :])
            nc.vector.reciprocal_approx_fast(drow[:], draw[:])
            nc.gpsimd.partition_broadcast(dinv[:], drow[:])
            nc.vector.tensor_mul(
                aT_sb[64 * par:64 * (par + 1),
                      N * hp + 512 * qb:N * hp + 512 * (qb + 1)],
                po[0:D, :], dinv[:])

        # queue V' projection as fillers for scores(u0)/scores(u1) slots
        for kc in range(NCH):
            fillers.extend(vproj_items(kc))

        P_cur = emit_scores_unit(0)

        pending_out = None
        for u in range(8):
            qb, hp = u // 2, u % 2
            po_e = popool.tile([128, 512], F32, tag="po", name=f"poe{u}")
            po_o = popool.tile([128, 512], F32, tag="po", name=f"poo{u}")
            if u < 7:
                # PV(u) singles interleaved e,o per kc
                for kc in range(NCH):
                    fillers.append(pv_item(u, P_cur, po_e, 0, kc))
                    fillers.append(pv_item(u, P_cur, po_o, 1, kc))
                if pending_out is not None:
                    oqb, ev = pending_out
                    for m in range(8):
                        fillers.append(out_item(oqb, ev, m))
                    fillers.append(out_flush(oqb, ev))
                    pending_out = None
                P_cur = emit_scores_unit(u + 1)
                # drain any leftover fillers (PV must finish before fin)
                while fillers:
                    fillers.pop(0)[2]()
                fin_head(u, po_e, 0)
                fin_head(u, po_o, 1)
            else:
                # tail: head-even chain, normalize it while head-odd runs
                for kc in range(NCH):
                    pv_item(u, P_cur, po_e, 0, kc)[2]()
                fin_head(u, po_e, 0)
                for kc in range(NCH):
                    pv_item(u, P_cur, po_o, 1, kc)[2]()
                fin_head(u, po_o, 1)
            if hp == 1:
                ev = evpool.tile([128, 8 * 512], BF16, tag="ev", name=f"ev{qb}")
                if u < 7:
                    pending_out = (qb, ev)
                else:
                    for m in range(8):
                        out_item(qb, ev, m)[2]()
                    out_flush(qb, ev)[2]()

    nc.compile()
    return nc


def _get_nc():
    if "nc" not in _CACHE:
        _CACHE["nc"] = build_nc()
    return _CACHE["nc"]


def _make_in_maps(q, k, v, Wq, Wk, Wv, Wo):
    bf = ml_dtypes.bfloat16
    q, k, v = np.asarray(q), np.asarray(k), np.asarray(v)
    Wq, Wk, Wv, Wo = (np.asarray(a) for a in (Wq, Wk, Wv, Wo))
    xqT = [np.ascontiguousarray(q[b].T).astype(bf) for b in range(B)]
    xkT = [np.ascontiguousarray(k[b].T).astype(bf) for b in range(B)]
    xvT = [np.ascontiguousarray(v[b].T).astype(bf) for b in range(B)]
    in_maps = []
    for c in range(8):
        b, g = c // 4, c % 4
        cs = slice(CL * g, CL * (g + 1))
        in_maps.append({
            "xqT": xqT[b], "xkT": xkT[b], "xvT": xvT[b],
            "wq": np.ascontiguousarray(Wq[:, cs]).astype(bf),
            "wk": np.ascontiguousarray(Wk[:, cs]).astype(bf),
            "wv": np.ascontiguousarray(Wv[:, cs]).astype(bf),
            "wo": np.ascontiguousarray(Wo[cs, :]).astype(bf),
        })
    return in_maps


def _run(inputs, trace=False, **kw):
    nc = _get_nc()
    in_maps = _make_in_maps(inputs["q"], inputs["k"], inputs["v"],
                            inputs["Wq"], inputs["Wk"], inputs["Wv"], inputs["Wo"])
    res = None
    for attempt in range(3):
        try:
            res = run_bass_kernel_spmd(nc, in_maps, core_ids=list(range(8)),
                                       trace=trace, **kw)
            break
        except Exception:
            if attempt == 2:
                raise
            import time
            time.sleep(2.0)
    out = np.empty((B, N, C), np.float32)
    for b in range(B):
        acc = res.results[4 * b]["outT"].astype(np.float32)
        for g in range(1, 4):
            acc += res.results[4 * b + g]["outT"].astype(np.float32)
        out[b] = acc.T
    return out, res


def kernel(**inputs) -> np.ndarray:
    out, _ = _run(inputs, trace=False)
    return out


# revision 18
# speedup vs baseline: 1.1269x; 1.0541x over previous
"""Distributed multi-head attention kernel for 8 TRN2 NeuronCores.

Problem: B=2, N=2048, C=1024, H=16 heads, D=64.
  out = softmax((q@Wq)(k@Wk)^T / sqrt(D)) @ (v@Wv) @ Wo   (biases are zero)

Sharding (hybrid): data-parallel over batch x tensor-parallel over heads.
Core c owns batch b=c//4 and head group g=c%4 (4 heads = 256 channels); it
projects Q/K/V for its own 256 columns over the full sequence, runs
attention for its 4 heads, applies its 256 rows of Wo and writes a PARTIAL
(1024, 2048) bf16 output.  The host sums the 4 partials per batch (the
out_proj all-reduce) and transposes.  Zero redundant FLOPs.

ScalarE is the critical path (~145us of exp ACTIVATE at (1024+352)/1.2 ns
per [128,1024] tile).  The emission strategy keeps it saturated:
  - batched DMAs (one descriptor per tensor half) so the head is
    transfer-bound, not dispatch-bound;
  - every non-score PE task (V' projection, PV, out_proj) is queued as
    small filler items interleaved into the score/exp slot stream, with
    slot gates to avoid in-order PE stalls on not-yet-ready deps;
  - out_proj of query block qb is spread into the unit after fin((qb,hp1));
  - the last unit's PV runs head-even first so its normalize overlaps the
    head-odd chain.
"""

import sys

sys.path.insert(0, "/opt/trn_rl_repo")

from contextlib import ExitStack

import numpy as np
import ml_dtypes

import concourse.bass as bass
import concourse.bacc as bacc
import concourse.mybir as mybir
import concourse.tile as tile
from concourse.bass_utils import run_bass_kernel_spmd

BF16 = mybir.dt.bfloat16
F32 = mybir.dt.float32
Exp = mybir.ActivationFunctionType.Exp

B, N, C = 2, 2048, 1024
H, D = 16, 64
HL, CL = 4, 256     # heads / channels owned per core
DV = D + 1          # V' columns per head incl. ones column
NCH = N // 128      # 16 key chunks
SCALE = 1.0 / np.sqrt(D)

_CACHE = {}


def build_nc():
    nc = bacc.Bacc("TRN2", target_bir_lowering=False, debug=False, num_devices=8)

    xqT = nc.declare_dram_parameter("xqT", [C, N], BF16, isOutput=False)
    xkT = nc.declare_dram_parameter("xkT", [C, N], BF16, isOutput=False)
    xvT = nc.declare_dram_parameter("xvT", [C, N], BF16, isOutput=False)
    wq = nc.declare_dram_parameter("wq", [C, CL], BF16, isOutput=False)
    wk = nc.declare_dram_parameter("wk", [C, CL], BF16, isOutput=False)
    wv = nc.declare_dram_parameter("wv", [C, CL], BF16, isOutput=False)
    wo = nc.declare_dram_parameter("wo", [CL, C], BF16, isOutput=False)
    outT = nc.declare_dram_parameter("outT", [C, N], BF16, isOutput=True)

    with tile.TileContext(nc) as tc, ExitStack() as top:
        res = top.enter_context(tc.tile_pool(name="res", bufs=1))
        qT_sb = res.tile([128, 2 * N], BF16, tag="qT")   # chunk hp at cols N*hp
        kT_sb = res.tile([128, 2 * N], BF16, tag="kT")
        v1_sb = res.tile([128, NCH * HL * DV], BF16, tag="v1")
        aT_sb = res.tile([128, 2 * N], BF16, tag="aT")
        v4 = v1_sb[:].rearrange("p (kc h x) -> p kc h x", kc=NCH, x=DV)

        def dma_in_grouped(dst_tile, src, blocks, tag):
            """One DMA per row-block: src rows [128*b0, 128*b1) -> dst col
            groups (global block index), dst laid out [128, nblk*width]."""
            width = src.shape[1]
            dst3 = dst_tile[:].rearrange("p (c m) -> p c m", m=width)
            for b0, b1 in blocks:
                nc.sync.dma_start(
                    out=dst3[:, b0:b1, :],
                    in_=src[128 * b0:128 * b1, :]
                    .rearrange("(c p) m -> p c m", p=128))

        # ---------------- phase A: K and Q projections ----------------
        with ExitStack() as ph:
            wpool = ph.enter_context(tc.tile_pool(name="wpool", bufs=1))
            xpool = ph.enter_context(tc.tile_pool(name="xpool", bufs=1))
            qkps = ph.enter_context(tc.tile_pool(name="qkps", bufs=4, space="PSUM"))

            # preload the exp table set so the first real ACTIVATE is cheap
            dummy = wpool.tile([1, 8], F32, tag="dummy")
            nc.vector.memset(dummy[:], 0.0)
            nc.scalar.activation(dummy[:], dummy[:], Exp, scale=1.0)

            wk_t = wpool.tile([128, 8 * CL], BF16, tag="wk")
            wq_t = wpool.tile([128, 8 * CL], BF16, tag="wq")
            # x tensors as quarter tiles (2 chunks each): dep tracking is
            # per-tile, so finer tiles let projections start on early data.
            xk_t = xpool.tile([128, 8 * N], BF16, tag="xk")
            xq_t = xpool.tile([128, 8 * N], BF16, tag="xq")
            dma_in_grouped(wk_t, wk, [(0, 8)], "wk")
            dma_in_grouped(wq_t, wq, [(0, 8)], "wq")
            dma_in_grouped(xk_t, xkT, [(0, 4), (4, 8)], "xk")
            dma_in_grouped(xq_t, xqT, [(0, 4), (4, 8)], "xq")
            wk3 = wk_t[:].rearrange("p (c m) -> p c m", m=CL)
            wq3 = wq_t[:].rearrange("p (c m) -> p c m", m=CL)
            xk_q = [xk_t]
            xq_q = [xq_t]

            def xsl(x_list, cc):
                return x_list[0][:].rearrange(
                    "p (c n) -> p c n", n=N)[:, cc, :]

            def proj_qk(dst_sb, w3, x_list, nm):
                for hp in range(2):
                    ps = [qkps.tile([128, 512], F32, tag="ps", name=f"{nm}ps{hp}_{qb}")
                          for qb in range(4)]
                    for cc in range(8):
                        xc = xsl(x_list, cc)
                        for qb in range(4):
                            nc.tensor.matmul(ps[qb][:],
                                             w3[:, cc, 128 * hp:128 * (hp + 1)],
                                             xc[:, 512 * qb:512 * (qb + 1)],
                                             start=(cc == 0), stop=(cc == 7))
                    for qb in range(4):
                        nc.vector.tensor_copy(
                            dst_sb[:, N * hp + 512 * qb:N * hp + 512 * (qb + 1)],
                            ps[qb][:])

            proj_qk(kT_sb, wk3, xk_q, "k")
            proj_qk(qT_sb, wq3, xq_q, "q")

        # ---------------- attention-phase pools ----------------
        P_pool = top.enter_context(tc.tile_pool(name="Ppool", bufs=3))
        spool = top.enter_context(tc.tile_pool(name="spool", bufs=2, space="PSUM"))
        popool = top.enter_context(tc.tile_pool(name="popool", bufs=2, space="PSUM"))
        wvpool = top.enter_context(tc.tile_pool(name="wvpool", bufs=1))
        xvpool = top.enter_context(tc.tile_pool(name="xvpool", bufs=1))
        wopool = top.enter_context(tc.tile_pool(name="wopool", bufs=1))
        evpool = top.enter_context(tc.tile_pool(name="evpool", bufs=2))
        dpool = top.enter_context(tc.tile_pool(name="dpool", bufs=2))

        wv_t = wvpool.tile([128, 8 * CL], BF16, tag="wv")
        xv_t = xvpool.tile([128, 8 * N], BF16, tag="xv")
        wo_t = wopool.tile([128, 2 * C], BF16, tag="wo")
        dma_in_grouped(wv_t, wv, [(0, 8)], "wv")
        dma_in_grouped(xv_t, xvT, [(0, 4), (4, 8)], "xv")
        xv_q = [xv_t]
        dma_in_grouped(wo_t, wo, [(0, 2)], "wo")
        wv3 = wv_t[:].rearrange("p (c m) -> p c m", m=CL)
        wo3 = wo_t[:].rearrange("p (c m) -> p c m", m=C)
        nc.vector.memset(v4[:, :, :, D:DV], 1.0)

        # --- filler queue: (cost, min_slot, emit_fn) consumed per slot ---
        fillers = []

        def emit_fillers(slot, budget=3):
            while fillers and budget > 0:
                cost, min_slot, fn = fillers[0]
                if slot < min_slot:
                    return
                fillers.pop(0)
                fn()
                budget -= cost

        def vproj_items(kc):
            def fn():
                ps = popool.tile([128, 512], F32, tag="out", name=f"vps{kc}")
                for cc in range(8):
                    nc.tensor.matmul(ps[:, 0:CL],
                                     xsl(xv_q, cc)[:, 128 * kc:128 * (kc + 1)],
                                     wv3[:, cc, :],
                                     start=(cc == 0), stop=(cc == 7))
                nc.vector.tensor_copy(
                    v4[:, kc, :, 0:D],
                    ps[:, 0:CL].rearrange("p (h d) -> p h d", d=D))
            return [(4, 8, fn)]

        def pv_item(u, Pp, po, par, kc):
            qb, hp = u // 2, u % 2

            def fn():
                nc.tensor.matmul(po[0:DV, :], v4[:, kc, 2 * hp + par, :],
                                 Pp[:, 1024 * kc + 512 * par:
                                    1024 * kc + 512 * (par + 1)],
                                 start=(kc == 0), stop=(kc == NCH - 1))
            return (1, 2, fn)

        def out_item(qb, ev, m):
            def fn():
                ps = popool.tile([128, 512], F32, tag="out", name=f"ops{qb}_{m}")
                for lc in range(2):
                    nc.tensor.matmul(ps[:], wo3[:, lc, 128 * m:128 * (m + 1)],
                                     aT_sb[:, N * lc + 512 * qb:
                                           N * lc + 512 * (qb + 1)],
                                     start=(lc == 0), stop=(lc == 1))
                ev3 = ev[:].rearrange("p (m n) -> p m n", n=512)
                nc.vector.tensor_copy(ev3[:, m, :], ps[:])
            return (2, 2, fn)

        def out_flush(qb, ev):
            def fn():
                nc.sync.dma_start(
                    out=outT.rearrange("(m p) n -> p m n", p=128)
                    [:, :, 512 * qb:512 * (qb + 1)],
                    in_=ev[:].rearrange("p (m n) -> p m n", n=512))
            return (0, 2, fn)

        # exp offload: these key chunks use the vector engine's Schraudolph
        # bit-trick (i16 = round(a*s + b) reinterpreted as bf16) instead of
        # ScalarE's ACTIVATE, balancing the two engines (~3% elementwise,
        # <0.1% on softmax output after num/denom cancellation).
        OFFLOAD_KC = ()
        LOG2E = float(np.log2(np.e))
        SCH_A = 128.0 * LOG2E * float(SCALE)
        SCH_B = 128.0 * (127.0 - 0.0436775)
        I16 = mybir.dt.int16

        def emit_scores_unit(u, with_fillers=True):
            qb, hp = u // 2, u % 2
            Pp = P_pool.tile([128, NCH * 1024], BF16, tag="P", name=f"P{u}")
            for kc in range(NCH):
                st = spool.tile([128, 1024], F32, tag="st", name=f"st{u}_{kc}")
                ksl = kT_sb[:, N * hp + 128 * kc:N * hp + 128 * (kc + 1)]
                qsl = qT_sb[:, N * hp + 512 * qb:N * hp + 512 * (qb + 1)]
                nc.tensor.matmul(st[:, 0:512], ksl[0:64, :], qsl[0:64, :],
                                 start=True, stop=True)
                nc.tensor.matmul(st[:, 512:1024], ksl[64:128, :], qsl[64:128, :],
                                 start=True, stop=True)
                pdst = Pp[:, 1024 * kc:1024 * (kc + 1)]
                if kc in OFFLOAD_KC:
                    nc.vector.tensor_scalar(pdst.bitcast(I16), st[:],
                                            SCH_A, SCH_B,
                                            mybir.AluOpType.mult,
                                            mybir.AluOpType.add)
                else:
                    nc.scalar.activation(pdst, st[:], Exp, scale=float(SCALE))
                if with_fillers:
                    emit_fillers(kc)
            return Pp

        def fin_head(u, po, par):
            qb, hp = u // 2, u % 2
            draw = dpool.tile([1, 512], F32, tag="draw", name=f"dr{u}_{par}")
            drow = dpool.tile([1, 512], F32, tag="drow", name=f"dw{u}_{par}")
            dinv = dpool.tile([64, 512], F32, tag="dinv", name=f"di{u}_{par}")
            nc.vector.tensor_copy(draw[:], po[D:DV, :])
            nc.vector.reciprocal_approx_fast(drow[:], draw[:])
            nc.gpsimd.partition_broadcast(dinv[:], drow[:])
            nc.vector.tensor_mul(
                aT_sb[64 * par:64 * (par + 1),
                      N * hp + 512 * qb:N * hp + 512 * (qb + 1)],
                po[0:D, :], dinv[:])

        # queue V' projection as fillers for scores(u0)/scores(u1) slots
        for kc in range(NCH):
            fillers.extend(vproj_items(kc))

        P_cur = emit_scores_unit(0)

        pending_out = None
        for u in range(8):
            qb, hp = u // 2, u % 2
            po_e = popool.tile([128, 512], F32, tag="po", name=f"poe{u}")
            po_o = popool.tile([128, 512], F32, tag="po", name=f"poo{u}")
            if u < 7:
                # PV(u) singles interleaved e,o per kc
                for kc in range(NCH):
                    fillers.append(pv_item(u, P_cur, po_e, 0, kc))
                    fillers.append(pv_item(u, P_cur, po_o, 1, kc))
                if pending_out is not None:
                    oqb, ev = pending_out
                    for m in range(8):
                        fillers.append(out_item(oqb, ev, m))
                    fillers.append(out_flush(oqb, ev))
                    pending_out = None
                P_cur = emit_scores_unit(u + 1)
                # drain any leftover fillers (PV must finish before fin)
                while fillers:
                    fillers.pop(0)[2]()
                fin_head(u, po_e, 0)
                fin_head(u, po_o, 1)
            else:
                # tail: head-even chain, normalize it while head-odd runs
                for kc in range(NCH):
                    pv_item(u, P_cur, po_e, 0, kc)[2]()
                fin_head(u, po_e, 0)
                for kc in range(NCH):
                    pv_item(u, P_cur, po_o, 1, kc)[2]()
                fin_head(u, po_o, 1)
            if hp == 1:
                ev = evpool.tile([128, 8 * 512], BF16, tag="ev", name=f"ev{qb}")
                if u < 7:
                    pending_out = (qb, ev)
                else:
                    for m in range(8):
                        out_item(qb, ev, m)[2]()
                    out_flush(qb, ev)[2]()

    nc.compile()
    return nc


def _get_nc():
    if "nc" not in _CACHE:
        _CACHE["nc"] = build_nc()
    return _CACHE["nc"]


def _make_in_maps(q, k, v, Wq, Wk, Wv, Wo):
    bf = ml_dtypes.bfloat16
    q, k, v = np.asarray(q), np.asarray(k), np.asarray(v)
    Wq, Wk, Wv, Wo = (np.asarray(a) for a in (Wq, Wk, Wv, Wo))
    xqT = [np.ascontiguousarray(q[b].T).astype(bf) for b in range(B)]
    xkT = [np.ascontiguousarray(k[b].T).astype(bf) for b in range(B)]
    xvT = [np.ascontiguousarray(v[b].T).astype(bf) for b in range(B)]
    in_maps = []
    for c in range(8):
        b, g = c // 4, c % 4
        cs = slice(CL * g, CL * (g + 1))
        in_maps.append({
            "xqT": xqT[b], "xkT": xkT[b], "xvT": xvT[b],
            "wq": np.ascontiguousarray(Wq[:, cs]).astype(bf),
            "wk": np.ascontiguousarray(Wk[:, cs]).astype(bf),
            "wv": np.ascontiguousarray(Wv[:, cs]).astype(bf),
            "wo": np.ascontiguousarray(Wo[cs, :]).astype(bf),
        })
    return in_maps


def _run(inputs, trace=False, **kw):
    nc = _get_nc()
    in_maps = _make_in_maps(inputs["q"], inputs["k"], inputs["v"],
                            inputs["Wq"], inputs["Wk"], inputs["Wv"], inputs["Wo"])
    res = None
    for attempt in range(3):
        try:
            res = run_bass_kernel_spmd(nc, in_maps, core_ids=list(range(8)),
                                       trace=trace, **kw)
            break
        except Exception:
            if attempt == 2:
                raise
            import time
            time.sleep(2.0)
    out = np.empty((B, N, C), np.float32)
    for b in range(B):
        acc = res.results[4 * b]["outT"].astype(np.float32)
        for g in range(1, 4):
            acc += res.results[4 * b + g]["outT"].astype(np.float32)
        out[b] = acc.T
    return out, res


def kernel(**inputs) -> np.ndarray:
    out, _ = _run(inputs, trace=False)
    return out
